# revision 60
# baseline (speedup 1.0000x reference)
"""Trainium2 Bass kernel for MQA attention (B=4, T=1024, D=2048, 16 q-heads, 1 kv-head).

Sharding: 8 cores = 4 batches x 2 head-groups (8 query heads each).

Structure (all phases TensorE-serial; the kernel is Tensor-bound, so the
program is shaped to keep the PE array continuously busy at the 2.4 GHz
ramped P-state):
  - warmup: dummy identity matmuls from ~7.5us so the PE P-state is fully
    ramped before the first real matmul (a cold PE runs 2x slow for ~5us).
  - chunk loop: k^T, v^T and q^T(head 0) projections fused per D-chunk, so
    compute tracks the x DMA. All are [H, tok]-producing 512-free matmuls
    (v as v^T then 8 transposes into [tok, H] vext tiles; this halves the
    old per-token-block v-proj stationary-reload cost).
  - per head: causal attention in transposed-logits layout ([k, q]) with the
    next head's q-projection matmuls interleaved into the logits->exp->PV
    pipeline slots so TensorE never idles while ScalarE runs exp.
    Softmax denominator rides as a fused ones-column of the PV rhs; no
    max-subtraction (logits bounded by construction). The diagonal d1 chunk
    only computes the valid 128-col half (causal trim); mask is one
    [128,384] multiply with a [tri|ones|tri] mask post-exp.
  - phase 3: output projection per 512-col chunk (c4-outer) so each chunk's
    cast/writeback overlaps the next chunk's matmuls; the last row-block is
    written back in 4 chunks to cut the tail.
Host sums the two partials per batch (the pair all-reduce) and stacks batches.

Matmul inputs are bf16 (f32 PSUM accumulation); rope tables f32; softmax
statistics and normalization stay f32. The q scaling H^-0.5 is folded into
wq on the host (rope is linear), so q and k share one cos/sin table pair.

The SPMD program is identical on all cores; only the data differs.
"""

import numpy as np
import ml_dtypes
import concourse.bass as bass
import concourse.mybir as mybir
from concourse import bacc
from concourse.tile import TileContext
from concourse.bass_utils import run_bass_kernel_spmd
from concourse.masks import make_identity
from contextlib import ExitStack

F32 = mybir.dt.float32
BF16 = mybir.dt.bfloat16
NP_BF16 = ml_dtypes.bfloat16

B, T, D, NH, HD = 4, 1024, 2048, 16, 128
HHD = HD // 2          # 64, rope half
NL = NH // 2           # 8 heads per core
DC = D // 128          # 16 contraction chunks
TT = T // 128          # 8 token tiles
EXPAD = 129            # PV rhs width: [v (128) | ones (1)]
N_WARM = 11            # warmup matmuls (512-free) before first real work

# Rope-pair interleave: the H dim of q/k is permuted (consistently in wq/wk
# columns, host-side) so each rope pair (f, f+64) sits 16 lanes apart within
# one 32-partition quadrant; the rotate-half becomes a stream_shuffle.
SHUF_MASK = list(range(16, 32)) + list(range(16))


def _rope(nc, out, pinb, cos, sin, tmp, stage):
    """RoPE in permuted [H, tok] layout, all-bf16 so the DVE runs in its
    2x/4x 16-bit mode (an f32/psum operand drops it to 1x, ~0.65us per
    [128,512] op -- measured). pinb: [128, W] bf16 sbuf (pre-cast from psum
    by ScalarE), cos: duplicated cos table, sin: sign-baked sin table (-sin
    on first-half lanes, +sin on second-half lanes), tmp/stage: bf16 scratch.
    out (bf16) = pinb * cos + shuffle16(pinb) * sin.
    """
    nc.vector.stream_shuffle(tmp, pinb, SHUF_MASK)
    nc.vector.tensor_mul(stage, pinb, cos)
    nc.vector.tensor_mul(tmp, tmp, sin)
    nc.vector.tensor_add(out, stage, tmp)


def build_nc():
    nc = bacc.Bacc("TRN2", target_bir_lowering=False, debug=False, num_devices=8)
    dt = F32
    xT_d = nc.dram_tensor("xT", [DC, 128, T], BF16, kind="ExternalInput").ap()
    wq_d = nc.dram_tensor("wq", [NL, 128, DC, HD], BF16, kind="ExternalInput").ap()
    wk_d = nc.dram_tensor("wk", [128, DC, HD], BF16, kind="ExternalInput").ap()
    wv_d = nc.dram_tensor("wv", [128, DC, HD], BF16, kind="ExternalInput").ap()
    wo_d = nc.dram_tensor("wo", [NL, 128, D], BF16, kind="ExternalInput").ap()
    cosk_d = nc.dram_tensor("cosk", [128, T], BF16, kind="ExternalInput").ap()
    sink_d = nc.dram_tensor("sink", [128, T], BF16, kind="ExternalInput").ap()
    tri_d = nc.dram_tensor("tri", [128, 384], BF16, kind="ExternalInput").ap()
    out_d = nc.dram_tensor("out", [T, D], BF16, kind="ExternalOutput").ap()

    with TileContext(nc) as tc, ExitStack() as ctx:
        singles = ctx.enter_context(tc.tile_pool(name="singles", bufs=1))

        # one tile per D-chunk so each chunk DMA unblocks compute immediately
        xTs = [singles.tile([128, T], BF16, name=f"xT{c}") for c in range(DC)]
        kT = singles.tile([128, T], BF16)          # roped k^T
        vTs = singles.tile([128, T], BF16)         # v^T staging (pre-transpose)
        vext = singles.tile([128, TT, EXPAD], BF16)  # v | ones column
        encT = singles.tile([128, NL, TT, 128], BF16)  # encoded^T per head, 2MB

        # warmup operands come from memsets (not the identity) and are
        # emitted first, so the PE P-state ramp starts ~0.8us earlier than
        # waiting for the iota/compare chain that builds `ident`.
        warmb = singles.tile([128, 512], BF16)
        warmw = singles.tile([128, 128], BF16)
        nc.vector.memset(warmw, 0.0)
        nc.vector.memset(warmb, 0.0)
        ident = singles.tile([128, 128], BF16)
        make_identity(nc, ident)

        # warm up the exp activation table set while DMAs land
        warm = singles.tile([128, 1], dt)
        warm2 = singles.tile([128, 1], dt)
        nc.vector.memset(warm, 0.0)
        nc.scalar.activation(out=warm2, in_=warm,
                             func=mybir.ActivationFunctionType.Exp)

        wk_sb = singles.tile([128, DC, HD], BF16)
        wv_sb = singles.tile([128, DC, HD], BF16)
        cosk = singles.tile([128, T], BF16)
        sink = singles.tile([128, T], BF16)
        tri = singles.tile([128, 384], BF16)
        wqp = ctx.enter_context(tc.tile_pool(name="wqp", bufs=NL))
        wop = ctx.enter_context(tc.tile_pool(name="wop", bufs=NL))
        wq_sbs = [wqp.tile([128, DC, HD], BF16, tag="wq", name=f"wq_t{n}")
                  for n in range(NL)]
        wo_sbs = [wop.tile([128, D], BF16, tag="wo", name=f"wo_t{n}")
                  for n in range(NL)]

        # Each HW dynamic queue (sync=q1, scalar=q10) pulls ~180GB/s when both
        # are active; the gpsimd queue is software-dynamic (~85GB/s) -- never
        # use it. A dma_start whose ring is full BLOCKS the issuing engine's
        # instruction stream, so the scalar queue gets ONLY the early
        # transfers (it must be free for the psum casts / vext copies / exps
        # from ~30us on) and sync carries the long tail (its only compute
        # duties are semaphores, as in the baseline). Everything is issued in
        # consumption order, first weights split into 4-chunk pieces and
        # x0/x1 into halves so the first k-proj matmul fires after ~256KB
        # (subtile deps make each matmul wait only on the piece it reads).
        # weight tensors go in HALVES (8 chunks / 2KB-per-partition rows --
        # smaller pieces mean 1KB DMA descriptors, which halve early DMA
        # throughput); x chunks stay whole for the same reason.
        def whalf(dst, src, p):
            return dst[:, 8 * p:8 * (p + 1), :], src[:, 8 * p:8 * (p + 1), :]

        nc.sync.dma_start(*whalf(wk_sb, wk_d, 0))
        nc.scalar.dma_start(*whalf(wv_sb, wv_d, 0))
        nc.sync.dma_start(out=xTs[0], in_=xT_d[0])
        nc.scalar.dma_start(out=xTs[1], in_=xT_d[1])
        nc.sync.dma_start(out=xTs[2], in_=xT_d[2])
        nc.scalar.dma_start(*whalf(wq_sbs[0], wq_d[0], 0))
        nc.sync.dma_start(out=xTs[4], in_=xT_d[4])
        nc.scalar.dma_start(out=xTs[3], in_=xT_d[3])
        nc.sync.dma_start(*whalf(wk_sb, wk_d, 1))
        nc.scalar.dma_start(out=xTs[5], in_=xT_d[5])
        nc.sync.dma_start(out=xTs[6], in_=xT_d[6])
        nc.scalar.dma_start(*whalf(wv_sb, wv_d, 1))
        nc.sync.dma_start(out=xTs[8], in_=xT_d[8])
        nc.scalar.dma_start(out=xTs[7], in_=xT_d[7])
        nc.sync.dma_start(out=xTs[10], in_=xT_d[10])
        nc.scalar.dma_start(*whalf(wq_sbs[0], wq_d[0], 1))
        nc.sync.dma_start(out=xTs[12], in_=xT_d[12])
        nc.scalar.dma_start(out=xTs[9], in_=xT_d[9])
        nc.sync.dma_start(out=xTs[13], in_=xT_d[13])
        nc.scalar.dma_start(out=xTs[11], in_=xT_d[11])
        nc.sync.dma_start(out=xTs[14], in_=xT_d[14])
        nc.scalar.dma_start(out=xTs[15], in_=xT_d[15])
        # rope tables (bf16) land right before the chunk-loop epilogue
        nc.sync.dma_start(out=cosk, in_=cosk_d)
        nc.sync.dma_start(out=sink, in_=sink_d)
        nc.sync.dma_start(out=tri, in_=tri_d)
        # remaining weights all on sync, in consumption order; the scalar
        # queue stays untouched from here on.
        for n in range(1, NL):
            nc.sync.dma_start(out=wq_sbs[n], in_=wq_d[n])
        for n in range(NL):
            nc.sync.dma_start(out=wo_sbs[n], in_=wo_d[n])

        # softmax-denominator ones column of every vext block, set once
        nc.vector.memset(vext[:, :, 128:129], 1.0)

        qtp = ctx.enter_context(tc.tile_pool(name="qtp", bufs=2))
        qT0 = qtp.tile([128, T], BF16, tag="qT", name="qT0")

        # P-state warmup: back-to-back dummy matmuls with no DMA deps so the
        # PE clock is ramped when wk/x0 land. Own scope so its bank is free
        # again before the transpose pool below allocates.
        with tc.tile_pool(name="pwarm", bufs=1, space="PSUM") as pwarm:
            wps = pwarm.tile([128, 512], dt)
            for _ in range(N_WARM):
                nc.tensor.matmul(wps, ident, warmb, start=True, stop=True)

        # ---- fused chunk loop: k^T, v^T, q0^T projections track the x DMA ----
        with tc.tile_pool(name="pk1", bufs=1, space="PSUM") as pk1, \
             tc.tile_pool(name="pv1", bufs=1, space="PSUM") as pv1, \
             tc.tile_pool(name="pq1", bufs=1, space="PSUM") as pq1, \
             tc.tile_pool(name="pt1", bufs=2, space="PSUM") as pt1, \
             tc.tile_pool(name="ktmp", bufs=2) as ktmp:
            pk = pk1.tile([128, 1024], dt)
            pv = pv1.tile([128, 1024], dt)
            pq = pq1.tile([128, 1024], dt)
            pkb = singles.tile([128, T], BF16, name="pkb")
            pqb = singles.tile([128, T], BF16, name="pqb")

            # start=True zeroes the whole 2KB psum bank ("zero region"), so
            # each bank gets exactly one start (its first matmul) and one
            # stop (its last); disjoint regions in between accumulate onto
            # the zeroed bank. q0 lags k/v by 2 chunks so its weight pieces
            # (interleaved mid-x on the scalar queue) never gate the loop.
            def q0_chunk(c):
                for half in range(2):
                    sl = slice(half * 512, (half + 1) * 512)
                    nc.tensor.matmul(pq[:, sl], wq_sbs[0][:, c, :],
                                     xTs[c][:, sl], start=(c == 0),
                                     stop=(c == DC - 1))

            for c in range(DC):
                st, sp = (c == 0), (c == DC - 1)
                for half in range(2):
                    sl = slice(half * 512, (half + 1) * 512)
                    nc.tensor.matmul(pk[:, sl], wk_sb[:, c, :], xTs[c][:, sl],
                                     start=st, stop=sp)
                    nc.tensor.matmul(pv[:, sl], wv_sb[:, c, :], xTs[c][:, sl],
                                     start=st, stop=sp)
                if c >= 2:
                    q0_chunk(c - 2)
            for c in range(DC - 2, DC):
                q0_chunk(c)
            # Epilogue engine split: ScalarE casts psum->bf16 (vTs + rope
            # inputs), VectorE ropes in pure bf16 (2x/4x DVE mode) with the
            # vext fills (psum->sbuf, GpSimd can't touch psum) slotted
            # between the first and second rope halves -- attention qb0 only
            # needs the h0 ropes, vext[:, 0:2] and kT chunks 0-1.
            for th in range(2):
                sl = slice(th * 512, (th + 1) * 512)
                nc.scalar.copy(out=vTs[:, sl], in_=pv[:, sl])

            def chunk_rope(th):
                sl = slice(th * 512, (th + 1) * 512)
                nc.scalar.copy(out=pkb[:, sl], in_=pk[:, sl])
                nc.scalar.copy(out=pqb[:, sl], in_=pq[:, sl])
                tmp = ktmp.tile([128, 512], BF16)
                stage = ktmp.tile([128, 512], BF16, tag="stage", name="kstage")
                _rope(nc, kT[:, sl], pkb[:, sl], cosk[:, sl], sink[:, sl],
                      tmp, stage)
                _rope(nc, qT0[:, sl], pqb[:, sl], cosk[:, sl], sink[:, sl],
                      tmp, stage)

            chunk_rope(0)
            # v^T -> vext [tok, H] via 8 transposes (stationary = vTs tile);
            # sbuf->sbuf DMA xbar transposes give wrong data on HW, so these
            # stay on TensorE with the copies on VectorE between rope halves.
            for tb in range(TT):
                ptt = pt1.tile([128, 128], BF16)
                nc.tensor.transpose(ptt, vTs[:, tb * 128:(tb + 1) * 128],
                                    ident)
                nc.vector.tensor_copy(out=vext[:, tb, 0:128], in_=ptt)
            chunk_rope(1)

        # ---- per-head causal attention. Heads 0-6 interleave the NEXT
        # head's q-projection into their logits->exp->PV pipeline slots so
        # TensorE never idles while ScalarE runs exp; head 7 interleaves
        # phase-3 output-projection chunks instead (gated so each chunk's
        # n=7 term follows its encT write). ----
        with tc.tile_pool(name="ropet", bufs=2) as ropet, \
             tc.tile_pool(name="expp", bufs=4) as expp, \
             tc.tile_pool(name="encp", bufs=4) as encp, \
             tc.tile_pool(name="recp", bufs=2) as recp, \
             tc.tile_pool(name="outp", bufs=2) as outp:

            def qproj_steps(n, qT, pq2):
                """Generator: emit head n's q-projection in small slices.
                Yields after every couple of matmuls so attention code can
                interleave these between its own TensorE ops."""
                for th in range(2):
                    sl = slice(th * 512, (th + 1) * 512)
                    pq = pq2.tile([128, 512], dt)
                    for c in range(DC):
                        nc.tensor.matmul(pq, wq_sbs[n][:, c, :], xTs[c][:, sl],
                                         start=(c == 0), stop=(c == DC - 1))
                        if c % 2 == 1:
                            yield
                    pb = ropet.tile([128, 512], BF16, tag="pb", name="pb")
                    nc.scalar.copy(out=pb, in_=pq)
                    tmp = ropet.tile([128, 512], BF16)
                    stage = ropet.tile([128, 512], BF16, tag="qstage",
                                       name="qstage")
                    _rope(nc, qT[:, sl], pb, cosk[:, sl], sink[:, sl], tmp,
                          stage)
                    yield
                while True:
                    yield

            def attn_head(n, qT, slot, qb_done, pl2, pe2, pt2):
                # The softmax-finalize (recip/scale/transpose/encT) of each
                # q-block is deferred into the NEXT q-block's first logits
                # group so the transposes follow long logits streams; the
                # deferred reads still precede the next PV's start=True
                # writes in emission order, so the shared pe banks are safe.
                pending = []

                def finalize():
                    while pending:
                        ts, pes = pending.pop(0)
                        rc = recp.tile([128, 1], dt)
                        nc.vector.reciprocal(rc, pes[:, 128:129])
                        en = encp.tile([128, 128], BF16)
                        nc.vector.tensor_scalar_mul(en, pes[:, 0:128], rc)
                        ptt = pt2.tile([128, 128], BF16)
                        nc.tensor.transpose(ptt, en, ident)
                        nc.vector.tensor_copy(out=encT[:, n, ts, :], in_=ptt)
                        slot()

                for qb in range(4):          # q blocks of 256 rows
                    R = qb * 256
                    d1 = 2 * qb + 1          # last (diagonal) chunk
                    d0 = d1 - 1              # diagonal chunk of sub0
                    pe0 = pe2.tile([128, 129], dt, tag="pe0", name="pe0")
                    pe1 = pe2.tile([128, 129], dt, tag="pe1", name="pe1")
                    # full-rect chunk pairs, then the trimmed diagonal group
                    groups = [(k0, k0 + 2, 512) for k0 in range(0, d0, 2)]
                    groups.append((d0, d1 + 1, 384))
                    for gi, (k0, k1, W) in enumerate(groups):
                        ps = pl2.tile([128, 512], dt)
                        # one bank: single start (zeroes it) on the first
                        # chunk, stop on the last; the second chunk
                        # accumulates into its zeroed region. The d1 chunk
                        # only computes the valid sub1 half (128 cols).
                        for kc in range(k0, k1):
                            o = (kc - k0) * 256
                            cw = 256 if kc < d1 else 128
                            qo = R if kc < d1 else R + 128
                            nc.tensor.matmul(ps[:, o:o + cw],
                                             kT[:, kc * 128:(kc + 1) * 128],
                                             qT[:, qo:qo + cw],
                                             start=(kc == k0),
                                             stop=(kc == k1 - 1))
                        if gi == 0:
                            finalize()
                        slot()
                        ex = expp.tile([128, 512], BF16)
                        nc.scalar.activation(
                            out=ex[:, 0:W], in_=ps[:, 0:W],
                            func=mybir.ActivationFunctionType.Exp)
                        if k1 == d1 + 1:
                            # [tri | ones | tri] mask on the diagonal group
                            nc.vector.tensor_mul(ex[:, 0:384], ex[:, 0:384],
                                                 tri)
                        slot()
                        for kc in range(k0, k1):
                            o = (kc - k0) * 256
                            if kc <= d0:
                                nc.tensor.matmul(pe0, ex[:, o:o + 128],
                                                 vext[:, kc, :],
                                                 start=(kc == 0),
                                                 stop=(kc == d0))
                            eo = o + 128 if kc < d1 else o
                            nc.tensor.matmul(pe1, ex[:, eo:eo + 128],
                                             vext[:, kc, :],
                                             start=(kc == 0), stop=(kc == d1))
                        slot()
                    pending.append((2 * qb, pe0))
                    pending.append((2 * qb + 1, pe1))
                    qb_done(qb)
                finalize()

            def phase3_units(po3):
                """Phase-3 output projection as resumable units: one psum
                chunk (8 head-matmuls + cast + writeback) per (ts, c4),
                yielding every half-chunk."""
                for ts in range(TT):
                    ob = outp.tile([128, 2048], BF16)
                    for c4 in range(4):
                        po = po3.tile([128, 512], dt)
                        for nn in range(NL):
                            nc.tensor.matmul(
                                po, encT[:, nn, ts, :],
                                wo_sbs[nn][:, c4 * 512:(c4 + 1) * 512],
                                start=(nn == 0), stop=(nn == NL - 1))
                            if nn == 3:
                                yield
                        co = c4 * 512
                        rsl = slice(ts * 128, (ts + 1) * 128)
                        if ts == TT - 1 and c4 == 3:
                            # very last chunk: split cast + writeback across
                            # both free engines/queues to shave the tail
                            nc.scalar.copy(out=ob[:, co:co + 256],
                                           in_=po[:, 0:256])
                            nc.vector.tensor_copy(out=ob[:, co + 256:co + 512],
                                                  in_=po[:, 256:512])
                            nc.scalar.dma_start(out=out_d[rsl, co:co + 256],
                                                in_=ob[:, co:co + 256])
                            nc.sync.dma_start(out=out_d[rsl, co + 256:co + 512],
                                              in_=ob[:, co + 256:co + 512])
                        elif ts == TT - 1:
                            # tail: ship each 512-chunk as soon as it's cast,
                            # on the scalar queue (idle and ring-empty here)
                            nc.vector.tensor_copy(out=ob[:, co:co + 512],
                                                  in_=po)
                            nc.scalar.dma_start(out=out_d[rsl, co:co + 512],
                                                in_=ob[:, co:co + 512])
                        else:
                            nc.vector.tensor_copy(out=ob[:, co:co + 512],
                                                  in_=po)
                        yield
                    if ts < TT - 1:
                        nc.sync.dma_start(
                            out=out_d[ts * 128:(ts + 1) * 128, :], in_=ob)

            qts = [qT0, None]
            with tc.tile_pool(name="pq2", bufs=2, space="PSUM") as pq2, \
                 tc.tile_pool(name="pl2", bufs=2, space="PSUM") as pl2, \
                 tc.tile_pool(name="pe2", bufs=1, space="PSUM") as pe2, \
                 tc.tile_pool(name="pt2", bufs=2, space="PSUM") as pt2:
                for n in range(NL - 1):
                    if n == 0:
                        qts[1] = qtp.tile([128, T], BF16, tag="qT",
                                          name="qT1")
                    nxt = qproj_steps(n + 1, qts[(n + 1) % 2], pq2)
                    attn_head(n, qts[n % 2], lambda: next(nxt),
                              lambda qb: None, pl2, pe2, pt2)
                    # drain leftover q-proj before the next head reads qT
                    for _ in range(40):
                        next(nxt)

            # head 7 + phase 3 share one psum scope (pq2 freed above)
            with tc.tile_pool(name="pl2b", bufs=2, space="PSUM") as pl2b, \
                 tc.tile_pool(name="pe2b", bufs=1, space="PSUM") as pe2b, \
                 tc.tile_pool(name="pt2b", bufs=1, space="PSUM") as pt2b, \
                 tc.tile_pool(name="po3", bufs=3, space="PSUM") as po3:
                p3 = phase3_units(po3)
                state = {"allowed": 0, "pulled": 0, "done": False}

                def slot7():
                    k = 0
                    while (not state["done"] and k < 2
                           and state["pulled"] < state["allowed"]):
                        try:
                            next(p3)
                        except StopIteration:
                            state["done"] = True
                            return
                        state["pulled"] += 1
                        k += 1

                def qb_done7(qb):
                    # finalize (and so encT[:,7,ts]) is deferred one q-block,
                    # so after qb only ts <= 2qb-1 is written; each ts is 8
                    # generator yields (4 chunks x 2)
                    state["allowed"] = (2 * qb) * 8

                attn_head(NL - 1, qts[1], slot7, qb_done7, pl2b, pe2b, pt2b)
                for _ in p3:     # drain the remaining phase-3 work
                    pass
    nc.compile()
    return nc


def make_in_maps(x, wq, wkv, wo, segment_pos, attn_mask):
    x = np.asarray(x, dtype=np.float32)
    wq = np.asarray(wq, dtype=np.float32)
    wkv = np.asarray(wkv, dtype=np.float32)
    wo = np.asarray(wo, dtype=np.float32)
    segment_pos = np.asarray(segment_pos)
    attn_mask = np.asarray(attn_mask)

    # rope-pair interleave permutation (see SHUF_MASK): lane j of quadrant qd
    # holds orig dim qd*16+(j%16) for lanes 0-15, 64+qd*16+(j%16) for 16-31.
    lanes = np.arange(HD)
    qd, lane = lanes // 32, lanes % 32
    f = qd * 16 + (lane % 16)
    perm = np.where(lane < 16, f, HHD + f)
    sgn = np.where(lane < 16, np.float32(-1.0), np.float32(1.0))

    def _pch(w):     # [D, H] -> [128, DC, H] with D = (c p)
        return np.ascontiguousarray(
            w.reshape(DC, 128, HD).transpose(1, 0, 2).astype(NP_BF16))

    wk = _pch(wkv[0, 0][:, perm])
    wv = _pch(wkv[1, 0])
    frac = (2.0 / HD) * np.arange(HHD, dtype=np.float32)
    timescale = (np.float32(10000.0) ** frac).astype(np.float32)
    scale = np.float32(HD ** -0.5)

    in_maps = []
    for c in range(8):
        b, g = c // 2, c % 2
        pos = segment_pos[b].astype(np.float32)
        sinus = pos[:, None] / timescale[None, :]          # [T, 64]
        cos = np.cos(sinus).astype(np.float32).T           # [64, T]
        sin = np.sin(sinus).astype(np.float32).T
        cosD = cos[f, :]                                   # [128, T]
        sinS = sgn[:, None] * sin[f, :]
        tri1 = attn_mask[b, :128, :128].T.astype(NP_BF16)  # [k, q] lower-left
        tri = np.ones((128, 384), dtype=NP_BF16)
        tri[:, 0:128] = tri1
        tri[:, 256:384] = tri1
        xT = np.ascontiguousarray(
            x[b].astype(NP_BF16).T.reshape(DC, 128, T))
        in_maps.append({
            "xT": xT,
            # q scaling folded into wq so q-rope can reuse the k tables
            "wq": np.stack([_pch(wq[g * NL + n][:, perm] * scale)
                            for n in range(NL)]),
            "wk": wk,
            "wv": wv,
            "wo": np.ascontiguousarray(
                wo[g * NL:(g + 1) * NL].astype(NP_BF16)),
            "cosk": np.ascontiguousarray(cosD.astype(NP_BF16)),
            "sink": np.ascontiguousarray(sinS.astype(NP_BF16)),
            "tri": tri,
        })
    return in_maps


_NC_CACHE = None


def kernel(**inputs):
    global _NC_CACHE
    if _NC_CACHE is None:
        _NC_CACHE = build_nc()
    nc = _NC_CACHE
    in_maps = make_in_maps(
        inputs["x"], inputs["wq"], inputs["wkv"], inputs["wo"],
        inputs["segment_pos"], inputs["attn_mask"])
    res = run_bass_kernel_spmd(nc, in_maps, core_ids=list(range(8)))
    out = np.empty((B, T, D), dtype=np.float32)
    for b in range(B):
        out[b] = (res.results[2 * b]["out"].astype(np.float32)
                  + res.results[2 * b + 1]["out"].astype(np.float32))
    return out


# revision 61
# speedup vs baseline: 1.1900x; 1.1900x over previous
"""Trainium2 Bass kernel for MQA attention (B=4, T=1024, D=2048, 16 q-heads, 1 kv-head).

Sharding: 8 cores = 4 batches x 2 head-groups (8 query heads each).

Structure (all phases TensorE-serial; the kernel is Tensor-bound, so the
program is shaped to keep the PE array continuously busy at the 2.4 GHz
ramped P-state):
  - warmup: dummy identity matmuls from ~7.5us so the PE P-state is fully
    ramped before the first real matmul (a cold PE runs 2x slow for ~5us).
  - chunk loop: k^T, v^T and q^T(head 0) projections fused per D-chunk, so
    compute tracks the x DMA. All are [H, tok]-producing 512-free matmuls
    (v as v^T then 8 transposes into [tok, H] vext tiles; this halves the
    old per-token-block v-proj stationary-reload cost).
  - per head: causal attention in transposed-logits layout ([k, q]) with the
    next head's q-projection matmuls interleaved into the logits->exp->PV
    pipeline slots so TensorE never idles while ScalarE runs exp.
    Softmax denominator rides as a fused ones-column of the PV rhs; no
    max-subtraction (logits bounded by construction). The diagonal d1 chunk
    only computes the valid 128-col half (causal trim); mask is one
    [128,384] multiply with a [tri|ones|tri] mask post-exp.
  - phase 3: output projection per 512-col chunk (c4-outer) so each chunk's
    cast/writeback overlaps the next chunk's matmuls; the last row-block is
    written back in 4 chunks to cut the tail.
Host sums the two partials per batch (the pair all-reduce) and stacks batches.

Matmul inputs are bf16 (f32 PSUM accumulation); rope tables f32; softmax
statistics and normalization stay f32. The q scaling H^-0.5 is folded into
wq on the host (rope is linear), so q and k share one cos/sin table pair.

The SPMD program is identical on all cores; only the data differs.
"""

import numpy as np
import ml_dtypes
import concourse.bass as bass
import concourse.mybir as mybir
from concourse import bacc
from concourse.tile import TileContext
from concourse.bass_utils import run_bass_kernel_spmd
from concourse.masks import make_identity
from contextlib import ExitStack

F32 = mybir.dt.float32
BF16 = mybir.dt.bfloat16
NP_BF16 = ml_dtypes.bfloat16

B, T, D, NH, HD = 4, 1024, 2048, 16, 128
HHD = HD // 2          # 64, rope half
NL = NH // 2           # 8 heads per core
DC = D // 128          # 16 contraction chunks
TT = T // 128          # 8 token tiles
EXPAD = 129            # PV rhs width: [v (128) | ones (1)]
N_WARM = 11            # warmup matmuls (512-free) before first real work

# Rope-pair interleave: the H dim of q/k is permuted (consistently in wq/wk
# columns, host-side) so each rope pair (f, f+64) sits 16 lanes apart within
# one 32-partition quadrant; the rotate-half becomes a stream_shuffle.
SHUF_MASK = list(range(16, 32)) + list(range(16))


def _rope(nc, out, pinb, cos, sin, tmp, stage):
    """RoPE in permuted [H, tok] layout, all-bf16 so the DVE runs in its
    2x/4x 16-bit mode (an f32/psum operand drops it to 1x, ~0.65us per
    [128,512] op -- measured). pinb: [128, W] bf16 sbuf (pre-cast from psum
    by ScalarE), cos: duplicated cos table, sin: sign-baked sin table (-sin
    on first-half lanes, +sin on second-half lanes), tmp/stage: bf16 scratch.
    out (bf16) = pinb * cos + shuffle16(pinb) * sin.
    """
    nc.vector.stream_shuffle(tmp, pinb, SHUF_MASK)
    nc.vector.tensor_mul(stage, pinb, cos)
    nc.vector.tensor_mul(tmp, tmp, sin)
    nc.vector.tensor_add(out, stage, tmp)


def build_nc():
    nc = bacc.Bacc("TRN2", target_bir_lowering=False, debug=False, num_devices=8)
    dt = F32
    xT_d = nc.dram_tensor("xT", [DC, 128, T], BF16, kind="ExternalInput").ap()
    wq_d = nc.dram_tensor("wq", [NL, 128, DC, HD], BF16, kind="ExternalInput").ap()
    wk_d = nc.dram_tensor("wk", [128, DC, HD], BF16, kind="ExternalInput").ap()
    wv_d = nc.dram_tensor("wv", [128, DC, HD], BF16, kind="ExternalInput").ap()
    wo_d = nc.dram_tensor("wo", [NL, 128, D], BF16, kind="ExternalInput").ap()
    cosk_d = nc.dram_tensor("cosk", [128, T], BF16, kind="ExternalInput").ap()
    sink_d = nc.dram_tensor("sink", [128, T], BF16, kind="ExternalInput").ap()
    tri_d = nc.dram_tensor("tri", [128, 384], BF16, kind="ExternalInput").ap()
    out_d = nc.dram_tensor("out", [T, D], BF16, kind="ExternalOutput").ap()

    with TileContext(nc) as tc, ExitStack() as ctx:
        singles = ctx.enter_context(tc.tile_pool(name="singles", bufs=1))

        # one tile per D-chunk so each chunk DMA unblocks compute immediately
        xTs = [singles.tile([128, T], BF16, name=f"xT{c}") for c in range(DC)]
        kT = singles.tile([128, T], BF16)          # roped k^T
        vTs = singles.tile([128, T], BF16)         # v^T staging (pre-transpose)
        vext = singles.tile([128, TT, EXPAD], BF16)  # v | ones column
        encT = singles.tile([128, NL, TT, 128], BF16)  # encoded^T per head, 2MB

        warmb = singles.tile([128, 512], BF16)
        nc.vector.memset(warmb, 0.0)
        ident = singles.tile([128, 128], BF16)
        make_identity(nc, ident)

        # warm up the exp activation table set while DMAs land
        warm = singles.tile([128, 1], dt)
        warm2 = singles.tile([128, 1], dt)
        nc.vector.memset(warm, 0.0)
        nc.scalar.activation(out=warm2, in_=warm,
                             func=mybir.ActivationFunctionType.Exp)

        wk_sb = singles.tile([128, DC, HD], BF16)
        wv_sb = singles.tile([128, DC, HD], BF16)
        cosk = singles.tile([128, T], BF16)
        sink = singles.tile([128, T], BF16)
        tri = singles.tile([128, 384], BF16)
        wqp = ctx.enter_context(tc.tile_pool(name="wqp", bufs=NL))
        wop = ctx.enter_context(tc.tile_pool(name="wop", bufs=NL))
        wq_sbs = [wqp.tile([128, DC, HD], BF16, tag="wq", name=f"wq_t{n}")
                  for n in range(NL)]
        wo_sbs = [wop.tile([128, D], BF16, tag="wo", name=f"wo_t{n}")
                  for n in range(NL)]

        # Each HW dynamic queue (sync=q1, scalar=q10) pulls ~180GB/s when both
        # are active; the gpsimd queue is software-dynamic (~85GB/s) -- never
        # use it. A dma_start whose ring is full BLOCKS the issuing engine's
        # instruction stream, so the scalar queue gets ONLY the early
        # transfers (it must be free for the psum casts / vext copies / exps
        # from ~30us on) and sync carries the long tail (its only compute
        # duties are semaphores, as in the baseline). Everything is issued in
        # consumption order, first weights split into 4-chunk pieces and
        # x0/x1 into halves so the first k-proj matmul fires after ~256KB
        # (subtile deps make each matmul wait only on the piece it reads).
        # weight tensors go in HALVES (8 chunks / 2KB-per-partition rows --
        # smaller pieces mean 1KB DMA descriptors, which halve early DMA
        # throughput); x chunks stay whole for the same reason.
        def whalf(dst, src, p):
            return dst[:, 8 * p:8 * (p + 1), :], src[:, 8 * p:8 * (p + 1), :]

        nc.sync.dma_start(*whalf(wk_sb, wk_d, 0))
        nc.scalar.dma_start(*whalf(wv_sb, wv_d, 0))
        nc.sync.dma_start(out=xTs[0], in_=xT_d[0])
        nc.scalar.dma_start(out=xTs[1], in_=xT_d[1])
        nc.sync.dma_start(out=xTs[2], in_=xT_d[2])
        nc.scalar.dma_start(*whalf(wq_sbs[0], wq_d[0], 0))
        nc.sync.dma_start(out=xTs[4], in_=xT_d[4])
        nc.scalar.dma_start(out=xTs[3], in_=xT_d[3])
        nc.sync.dma_start(*whalf(wk_sb, wk_d, 1))
        nc.scalar.dma_start(out=xTs[5], in_=xT_d[5])
        nc.sync.dma_start(out=xTs[6], in_=xT_d[6])
        nc.scalar.dma_start(*whalf(wv_sb, wv_d, 1))
        nc.sync.dma_start(out=xTs[8], in_=xT_d[8])
        nc.scalar.dma_start(out=xTs[7], in_=xT_d[7])
        nc.sync.dma_start(out=xTs[10], in_=xT_d[10])
        nc.scalar.dma_start(*whalf(wq_sbs[0], wq_d[0], 1))
        nc.sync.dma_start(out=xTs[12], in_=xT_d[12])
        nc.scalar.dma_start(out=xTs[9], in_=xT_d[9])
        nc.sync.dma_start(out=xTs[13], in_=xT_d[13])
        nc.scalar.dma_start(out=xTs[11], in_=xT_d[11])
        nc.sync.dma_start(out=xTs[14], in_=xT_d[14])
        nc.scalar.dma_start(out=xTs[15], in_=xT_d[15])
        # rope tables (bf16) land right before the chunk-loop epilogue
        nc.sync.dma_start(out=cosk, in_=cosk_d)
        nc.sync.dma_start(out=sink, in_=sink_d)
        nc.sync.dma_start(out=tri, in_=tri_d)
        # remaining weights all on sync, in consumption order; the scalar
        # queue stays untouched from here on.
        for n in range(1, NL):
            nc.sync.dma_start(out=wq_sbs[n], in_=wq_d[n])
        for n in range(NL):
            nc.sync.dma_start(out=wo_sbs[n], in_=wo_d[n])

        # softmax-denominator ones column of every vext block, set once
        nc.vector.memset(vext[:, :, 128:129], 1.0)

        qtp = ctx.enter_context(tc.tile_pool(name="qtp", bufs=2))
        qT0 = qtp.tile([128, T], BF16, tag="qT", name="qT0")

        # P-state warmup: back-to-back dummy matmuls with no DMA deps so the
        # PE clock is ramped when wk/x0 land. Own scope so its bank is free
        # again before the transpose pool below allocates.
        with tc.tile_pool(name="pwarm", bufs=1, space="PSUM") as pwarm:
            wps = pwarm.tile([128, 512], dt)
            for _ in range(N_WARM):
                nc.tensor.matmul(wps, ident, warmb, start=True, stop=True)

        # ---- fused chunk loop: k^T, v^T, q0^T projections track the x DMA ----
        with tc.tile_pool(name="pk1", bufs=1, space="PSUM") as pk1, \
             tc.tile_pool(name="pv1", bufs=1, space="PSUM") as pv1, \
             tc.tile_pool(name="pq1", bufs=1, space="PSUM") as pq1, \
             tc.tile_pool(name="pt1", bufs=2, space="PSUM") as pt1, \
             tc.tile_pool(name="ktmp", bufs=2) as ktmp:
            pk = pk1.tile([128, 1024], dt)
            pv = pv1.tile([128, 1024], dt)
            pq = pq1.tile([128, 1024], dt)
            pkb = singles.tile([128, T], BF16, name="pkb")
            pqb = singles.tile([128, T], BF16, name="pqb")

            # start=True zeroes the whole 2KB psum bank ("zero region"), so
            # each bank gets exactly one start (its first matmul) and one
            # stop (its last); disjoint regions in between accumulate onto
            # the zeroed bank. q0 lags k/v by 2 chunks so its weight pieces
            # (interleaved mid-x on the scalar queue) never gate the loop.
            def q0_chunk(c):
                for half in range(2):
                    sl = slice(half * 512, (half + 1) * 512)
                    nc.tensor.matmul(pq[:, sl], wq_sbs[0][:, c, :],
                                     xTs[c][:, sl], start=(c == 0),
                                     stop=(c == DC - 1))

            for c in range(DC):
                st, sp = (c == 0), (c == DC - 1)
                for half in range(2):
                    sl = slice(half * 512, (half + 1) * 512)
                    nc.tensor.matmul(pk[:, sl], wk_sb[:, c, :], xTs[c][:, sl],
                                     start=st, stop=sp)
                    nc.tensor.matmul(pv[:, sl], wv_sb[:, c, :], xTs[c][:, sl],
                                     start=st, stop=sp)
                if c >= 2:
                    q0_chunk(c - 2)
            for c in range(DC - 2, DC):
                q0_chunk(c)
            # Epilogue engine split: ScalarE casts psum->bf16 (vTs + rope
            # inputs), VectorE ropes in pure bf16 (2x/4x DVE mode) with the
            # vext fills (psum->sbuf, GpSimd can't touch psum) slotted
            # between the first and second rope halves -- attention qb0 only
            # needs the h0 ropes, vext[:, 0:2] and kT chunks 0-1.
            for th in range(2):
                sl = slice(th * 512, (th + 1) * 512)
                nc.scalar.copy(out=vTs[:, sl], in_=pv[:, sl])

            def chunk_rope(th):
                sl = slice(th * 512, (th + 1) * 512)
                nc.scalar.copy(out=pkb[:, sl], in_=pk[:, sl])
                nc.scalar.copy(out=pqb[:, sl], in_=pq[:, sl])
                tmp = ktmp.tile([128, 512], BF16)
                stage = ktmp.tile([128, 512], BF16, tag="stage", name="kstage")
                _rope(nc, kT[:, sl], pkb[:, sl], cosk[:, sl], sink[:, sl],
                      tmp, stage)
                _rope(nc, qT0[:, sl], pqb[:, sl], cosk[:, sl], sink[:, sl],
                      tmp, stage)

            chunk_rope(0)
            # v^T -> vext [tok, H] via 8 transposes (stationary = vTs tile);
            # sbuf->sbuf DMA xbar transposes give wrong data on HW, so these
            # stay on TensorE with the copies on VectorE between rope halves.
            for tb in range(TT):
                ptt = pt1.tile([128, 128], BF16)
                nc.tensor.transpose(ptt, vTs[:, tb * 128:(tb + 1) * 128],
                                    ident)
                nc.vector.tensor_copy(out=vext[:, tb, 0:128], in_=ptt)
            chunk_rope(1)

        # ---- per-head causal attention. Heads 0-6 interleave the NEXT
        # head's q-projection into their logits->exp->PV pipeline slots so
        # TensorE never idles while ScalarE runs exp; head 7 interleaves
        # phase-3 output-projection chunks instead (gated so each chunk's
        # n=7 term follows its encT write). ----
        with tc.tile_pool(name="ropet", bufs=2) as ropet, \
             tc.tile_pool(name="expp", bufs=4) as expp, \
             tc.tile_pool(name="encp", bufs=4) as encp, \
             tc.tile_pool(name="recp", bufs=2) as recp, \
             tc.tile_pool(name="outp", bufs=2) as outp:

            def qproj_steps(n, qT, pq2):
                """Generator: emit head n's q-projection in small slices.
                Yields after every couple of matmuls so attention code can
                interleave these between its own TensorE ops."""
                for th in range(2):
                    sl = slice(th * 512, (th + 1) * 512)
                    pq = pq2.tile([128, 512], dt)
                    for c in range(DC):
                        nc.tensor.matmul(pq, wq_sbs[n][:, c, :], xTs[c][:, sl],
                                         start=(c == 0), stop=(c == DC - 1))
                        if c % 2 == 1:
                            yield
                    pb = ropet.tile([128, 512], BF16, tag="pb", name="pb")
                    nc.scalar.copy(out=pb, in_=pq)
                    tmp = ropet.tile([128, 512], BF16)
                    stage = ropet.tile([128, 512], BF16, tag="qstage",
                                       name="qstage")
                    _rope(nc, qT[:, sl], pb, cosk[:, sl], sink[:, sl], tmp,
                          stage)
                    yield
                while True:
                    yield

            def attn_head(n, qT, slot, qb_done, pl2, pe2, pt2):
                # The softmax-finalize (recip/scale/transpose/encT) of each
                # q-block is deferred into the NEXT q-block's first logits
                # group so the transposes follow long logits streams; the
                # deferred reads still precede the next PV's start=True
                # writes in emission order, so the shared pe banks are safe.
                pending = []

                def finalize():
                    while pending:
                        ts, pes = pending.pop(0)
                        rc = recp.tile([128, 1], dt)
                        nc.vector.reciprocal(rc, pes[:, 128:129])
                        en = encp.tile([128, 128], BF16)
                        nc.vector.tensor_scalar_mul(en, pes[:, 0:128], rc)
                        ptt = pt2.tile([128, 128], BF16)
                        nc.tensor.transpose(ptt, en, ident)
                        nc.vector.tensor_copy(out=encT[:, n, ts, :], in_=ptt)
                        slot()

                for qb in range(4):          # q blocks of 256 rows
                    R = qb * 256
                    d1 = 2 * qb + 1          # last (diagonal) chunk
                    d0 = d1 - 1              # diagonal chunk of sub0
                    pe0 = pe2.tile([128, 129], dt, tag="pe0", name="pe0")
                    pe1 = pe2.tile([128, 129], dt, tag="pe1", name="pe1")
                    # full-rect chunk pairs, then the trimmed diagonal group
                    groups = [(k0, k0 + 2, 512) for k0 in range(0, d0, 2)]
                    groups.append((d0, d1 + 1, 384))
                    for gi, (k0, k1, W) in enumerate(groups):
                        ps = pl2.tile([128, 512], dt)
                        # one bank: single start (zeroes it) on the first
                        # chunk, stop on the last; the second chunk
                        # accumulates into its zeroed region. The d1 chunk
                        # only computes the valid sub1 half (128 cols).
                        for kc in range(k0, k1):
                            o = (kc - k0) * 256
                            cw = 256 if kc < d1 else 128
                            qo = R if kc < d1 else R + 128
                            nc.tensor.matmul(ps[:, o:o + cw],
                                             kT[:, kc * 128:(kc + 1) * 128],
                                             qT[:, qo:qo + cw],
                                             start=(kc == k0),
                                             stop=(kc == k1 - 1))
                        if gi == 0:
                            finalize()
                        slot()
                        ex = expp.tile([128, 512], BF16)
                        nc.scalar.activation(
                            out=ex[:, 0:W], in_=ps[:, 0:W],
                            func=mybir.ActivationFunctionType.Exp)
                        if k1 == d1 + 1:
                            # [tri | ones | tri] mask on the diagonal group
                            nc.vector.tensor_mul(ex[:, 0:384], ex[:, 0:384],
                                                 tri)
                        slot()
                        for kc in range(k0, k1):
                            o = (kc - k0) * 256
                            if kc <= d0:
                                nc.tensor.matmul(pe0, ex[:, o:o + 128],
                                                 vext[:, kc, :],
                                                 start=(kc == 0),
                                                 stop=(kc == d0))
                            eo = o + 128 if kc < d1 else o
                            nc.tensor.matmul(pe1, ex[:, eo:eo + 128],
                                             vext[:, kc, :],
                                             start=(kc == 0), stop=(kc == d1))
                        slot()
                    pending.append((2 * qb, pe0))
                    pending.append((2 * qb + 1, pe1))
                    qb_done(qb)
                finalize()

            def phase3_units(po3):
                """Phase-3 output projection as resumable units: one psum
                chunk (8 head-matmuls + cast + writeback) per (ts, c4),
                yielding every half-chunk."""
                for ts in range(TT):
                    ob = outp.tile([128, 2048], BF16)
                    for c4 in range(4):
                        po = po3.tile([128, 512], dt)
                        for nn in range(NL):
                            nc.tensor.matmul(
                                po, encT[:, nn, ts, :],
                                wo_sbs[nn][:, c4 * 512:(c4 + 1) * 512],
                                start=(nn == 0), stop=(nn == NL - 1))
                            if nn == 3:
                                yield
                        co = c4 * 512
                        rsl = slice(ts * 128, (ts + 1) * 128)
                        if ts == TT - 1 and c4 == 3:
                            # very last chunk: split cast + writeback across
                            # both free engines/queues to shave the tail
                            nc.scalar.copy(out=ob[:, co:co + 256],
                                           in_=po[:, 0:256])
                            nc.vector.tensor_copy(out=ob[:, co + 256:co + 512],
                                                  in_=po[:, 256:512])
                            nc.scalar.dma_start(out=out_d[rsl, co:co + 256],
                                                in_=ob[:, co:co + 256])
                            nc.sync.dma_start(out=out_d[rsl, co + 256:co + 512],
                                              in_=ob[:, co + 256:co + 512])
                        elif ts == TT - 1:
                            # tail: ship each 512-chunk as soon as it's cast,
                            # on the scalar queue (idle and ring-empty here)
                            nc.vector.tensor_copy(out=ob[:, co:co + 512],
                                                  in_=po)
                            nc.scalar.dma_start(out=out_d[rsl, co:co + 512],
                                                in_=ob[:, co:co + 512])
                        else:
                            nc.vector.tensor_copy(out=ob[:, co:co + 512],
                                                  in_=po)
                        yield
                    if ts < TT - 1:
                        nc.sync.dma_start(
                            out=out_d[ts * 128:(ts + 1) * 128, :], in_=ob)

            qts = [qT0, None]
            with tc.tile_pool(name="pq2", bufs=2, space="PSUM") as pq2, \
                 tc.tile_pool(name="pl2", bufs=2, space="PSUM") as pl2, \
                 tc.tile_pool(name="pe2", bufs=1, space="PSUM") as pe2, \
                 tc.tile_pool(name="pt2", bufs=2, space="PSUM") as pt2:
                for n in range(NL - 1):
                    if n == 0:
                        qts[1] = qtp.tile([128, T], BF16, tag="qT",
                                          name="qT1")
                    nxt = qproj_steps(n + 1, qts[(n + 1) % 2], pq2)
                    attn_head(n, qts[n % 2], lambda: next(nxt),
                              lambda qb: None, pl2, pe2, pt2)
                    # drain leftover q-proj before the next head reads qT
                    for _ in range(40):
                        next(nxt)

            # head 7 + phase 3 share one psum scope (pq2 freed above)
            with tc.tile_pool(name="pl2b", bufs=2, space="PSUM") as pl2b, \
                 tc.tile_pool(name="pe2b", bufs=1, space="PSUM") as pe2b, \
                 tc.tile_pool(name="pt2b", bufs=1, space="PSUM") as pt2b, \
                 tc.tile_pool(name="po3", bufs=3, space="PSUM") as po3:
                p3 = phase3_units(po3)
                state = {"allowed": 0, "pulled": 0, "done": False}

                def slot7():
                    k = 0
                    while (not state["done"] and k < 2
                           and state["pulled"] < state["allowed"]):
                        try:
                            next(p3)
                        except StopIteration:
                            state["done"] = True
                            return
                        state["pulled"] += 1
                        k += 1

                def qb_done7(qb):
                    # finalize (and so encT[:,7,ts]) is deferred one q-block,
                    # so after qb only ts <= 2qb-1 is written; each ts is 8
                    # generator yields (4 chunks x 2)
                    state["allowed"] = (2 * qb) * 8

                attn_head(NL - 1, qts[1], slot7, qb_done7, pl2b, pe2b, pt2b)
                for _ in p3:     # drain the remaining phase-3 work
                    pass
    nc.compile()
    return nc


def make_in_maps(x, wq, wkv, wo, segment_pos, attn_mask):
    x = np.asarray(x, dtype=np.float32)
    wq = np.asarray(wq, dtype=np.float32)
    wkv = np.asarray(wkv, dtype=np.float32)
    wo = np.asarray(wo, dtype=np.float32)
    segment_pos = np.asarray(segment_pos)
    attn_mask = np.asarray(attn_mask)

    # rope-pair interleave permutation (see SHUF_MASK): lane j of quadrant qd
    # holds orig dim qd*16+(j%16) for lanes 0-15, 64+qd*16+(j%16) for 16-31.
    lanes = np.arange(HD)
    qd, lane = lanes // 32, lanes % 32
    f = qd * 16 + (lane % 16)
    perm = np.where(lane < 16, f, HHD + f)
    sgn = np.where(lane < 16, np.float32(-1.0), np.float32(1.0))

    def _pch(w):     # [D, H] -> [128, DC, H] with D = (c p)
        return np.ascontiguousarray(
            w.reshape(DC, 128, HD).transpose(1, 0, 2).astype(NP_BF16))

    wk = _pch(wkv[0, 0][:, perm])
    wv = _pch(wkv[1, 0])
    frac = (2.0 / HD) * np.arange(HHD, dtype=np.float32)
    timescale = (np.float32(10000.0) ** frac).astype(np.float32)
    scale = np.float32(HD ** -0.5)

    in_maps = []
    for c in range(8):
        b, g = c // 2, c % 2
        pos = segment_pos[b].astype(np.float32)
        sinus = pos[:, None] / timescale[None, :]          # [T, 64]
        cos = np.cos(sinus).astype(np.float32).T           # [64, T]
        sin = np.sin(sinus).astype(np.float32).T
        cosD = cos[f, :]                                   # [128, T]
        sinS = sgn[:, None] * sin[f, :]
        tri1 = attn_mask[b, :128, :128].T.astype(NP_BF16)  # [k, q] lower-left
        tri = np.ones((128, 384), dtype=NP_BF16)
        tri[:, 0:128] = tri1
        tri[:, 256:384] = tri1
        xT = np.ascontiguousarray(
            x[b].astype(NP_BF16).T.reshape(DC, 128, T))
        in_maps.append({
            "xT": xT,
            # q scaling folded into wq so q-rope can reuse the k tables
            "wq": np.stack([_pch(wq[g * NL + n][:, perm] * scale)
                            for n in range(NL)]),
            "wk": wk,
            "wv": wv,
            "wo": np.ascontiguousarray(
                wo[g * NL:(g + 1) * NL].astype(NP_BF16)),
            "cosk": np.ascontiguousarray(cosD.astype(NP_BF16)),
            "sink": np.ascontiguousarray(sinS.astype(NP_BF16)),
            "tri": tri,
        })
    return in_maps


_NC_CACHE = None


def kernel(**inputs):
    global _NC_CACHE
    if _NC_CACHE is None:
        _NC_CACHE = build_nc()
    nc = _NC_CACHE
    in_maps = make_in_maps(
        inputs["x"], inputs["wq"], inputs["wkv"], inputs["wo"],
        inputs["segment_pos"], inputs["attn_mask"])
    res = run_bass_kernel_spmd(nc, in_maps, core_ids=list(range(8)))
    out = np.empty((B, T, D), dtype=np.float32)
    for b in range(B):
        out[b] = (res.results[2 * b]["out"].astype(np.float32)
                  + res.results[2 * b + 1]["out"].astype(np.float32))
    return out
